# revision 75
# baseline (speedup 1.0000x reference)
"""Bahdanau 'concat' attention fused kernel for Trainium2, SPMD over 8 cores.

Math (per batch b, decoder position o, encoder position i):
    dp[k, (b,o)] = sum_h Wd[k,h] * dec[o,b,h]           (PE)
    ep[k, i]     = sum_h We[k,h] * enc[i,b,h]           (PE, via on-chip enc^T)
    t[k, i]      = tanh(ep[k,i] + dp[k,(b,o)] + bias[k])  (ACT / Pool+DVE pre-add)
    s[(b,o), i]  = sum_k v[k] * t[k, i]                 (PE, masked-column weights)
    w = softmax_i(s)                                    (ACT exp + accum_out; no
                                                         max-sub: |s| <= ||v||_1 ~ 5)
    out[o, b, h] = sum_i w[(b,o), i] * enc[i,b,h]       (PE, weights^T as stationary)

Sharding: data-parallel over OUT_LEN (o) across the 8 cores - 16 rows each; the
softmax is over i only, so no collectives are needed. enc and the tiny params
are replicated; dec is sliced per core.

The v-dot-over-partitions uses a masked stationary operand: a [128, 2J-1] strip
with v in column J-1 and zeros elsewhere. Slicing [J-1-j : 2J-1-j] puts v in
output row j and zeros in all other rows, so every (b,o) pair accumulates its
score row into one [64, 1024] PSUM tile with no partition-offset tricks.

The kernel is ACT-throughput-bound (67M tanh evaluations at 128 lanes/1.2GHz
= 54.6us/core floor). The per-batch schedule balances three ways of adding the
per-(b,o) bias before tanh: directly via ACT's per-partition bias operand
("A" tiles), or pre-added on the Pool/DVE engines and tanh'd in big grouped
ACTIVATEs that amortize the ~350-cycle instruction overhead ("G" tiles).
"""

import numpy as np
from contextlib import ExitStack

import concourse.bass as bass
import concourse.bacc as bacc
import concourse.tile as tile
from concourse import masks, mybir
from concourse.bass_utils import run_bass_kernel_spmd

OUT_LEN, IN_LEN, BATCH, HID = 128, 1024, 4, 128
N_CORES = 8
O_SHARD = OUT_LEN // N_CORES          # 16 decoder rows per core
J = BATCH * O_SHARD                   # 64 (b,o) pairs per core
NCH = IN_LEN // 128                   # 8 i-chunks
F32 = mybir.dt.float32
F32R = mybir.dt.float32r              # fast PE mode (TF32-like); sim == fp32

AF = mybir.ActivationFunctionType

# Per-batch tile schedule: ("A", [o..]) = tanh with per-partition bias on ACT;
# ("G", [(o, 'p'|'d'), ..]) = bias pre-add on Pool/DVE then one grouped tanh.
# b=0/b=1 keep A tiles up front so ACT starts before the pre-add pipeline has
# spun up; b=3 ends with A tiles so the final score matmuls trail less.
_G = lambda o0, kinds: ("G", [(o0 + i, k) for i, k in enumerate(kinds)])
SCHEDS = [
    [("A", [0, 1, 2, 3]), _G(4, "ddpp"), _G(8, "ppdd"), _G(12, "ppdd")],
    [("A", [0, 1]), _G(2, "ppdddpp" + "p"), _G(10, "ppdd" + "pd")],
    [_G(0, "pppddppd"), _G(8, "pppddppd")],
    [_G(0, "ppdd"), _G(4, "ppdd"), _G(8, "ppdd"), ("A", [12, 13, 14, 15])],
]

for _sched in SCHEDS:
    _os = [o for kind, m in _sched for o in (m if kind == "A" else [x for x, _ in m])]
    assert sorted(_os) == list(range(O_SHARD)), _os

_program_cache = {}


def build_program():
    if "nc" in _program_cache:
        return _program_cache["nc"]

    nc = bacc.Bacc(None, target_bir_lowering=False)
    # Small params packed into one tensor -> one DMA -> matmuls that read
    # them carry a single DMA-queue wait (the LDWEIGHTS slot allows only one).
    # Layout along free dim: wdt[0:128] | wet[128:256] | dect[256:320] |
    # biascol[320:321]. vstrip ships separately so this startup-critical DMA
    # stays small.
    enc_d = nc.dram_tensor("enc", [IN_LEN, BATCH * HID], F32, kind="ExternalInput")
    params_d = nc.dram_tensor("params", [HID, 321], F32, kind="ExternalInput")
    vstrip_d = nc.dram_tensor("vstrip", [HID, 2 * J - 1], F32, kind="ExternalInput")
    # raw [j, (b,h)] context block; the host picks the b==b(j) slices (unshard)
    out_d = nc.dram_tensor("out", [J, BATCH * HID], F32, kind="ExternalOutput")

    with ExitStack() as ctx:
        tc = ctx.enter_context(tile.TileContext(nc))
        singles = ctx.enter_context(tc.tile_pool(name="singles", bufs=1))
        enc_pool = ctx.enter_context(tc.tile_pool(name="encp", bufs=1))
        encT_pool = ctx.enter_context(tc.tile_pool(name="encT", bufs=2))
        tanh_pool = ctx.enter_context(tc.tile_pool(name="tanh", bufs=4))
        encr_pool = ctx.enter_context(tc.tile_pool(name="encr", bufs=1))
        epsb_pool = ctx.enter_context(tc.tile_pool(name="epsb", bufs=2))
        pre_pool = ctx.enter_context(tc.tile_pool(name="pre", bufs=2))
        tanhb_pool = ctx.enter_context(tc.tile_pool(name="tanhb", bufs=2))
        wt_pool = ctx.enter_context(tc.tile_pool(name="wt", bufs=2))
        ep_pool = ctx.enter_context(tc.tile_pool(name="ep", bufs=2, space="PSUM"))
        sc_pool = ctx.enter_context(tc.tile_pool(name="sc", bufs=1, space="PSUM"))
        tp_pool = ctx.enter_context(tc.tile_pool(name="tp", bufs=2, space="PSUM"))

        # One DMA per batch column-slice: b=0's whole enc slice lands first so
        # its transposes/ep/tanh start ~5us earlier than a chunk-ordered load.
        # params goes second: the b0 transpose chain needs only enc + the
        # gpsimd-built identity, while dp/dpb (params consumers) have slack.
        params_sb = singles.tile([HID, 321], F32, tag="params")
        nc.sync.dma_start(out=params_sb[:], in_=params_d[:, :])
        vstrip_sb_t = singles.tile([HID, 2 * J - 1], F32, tag="vstrip")
        encB = []
        for b in range(BATCH):
            t = enc_pool.tile([128, NCH, HID], F32, tag=f"encB{b}")
            if b == 0:
                # b0 split in halves: its first transposes start ~1.5us sooner
                hc = NCH // 2
                for half in range(2):
                    nc.sync.dma_start(
                        out=t[:, half * hc : (half + 1) * hc, :],
                        in_=enc_d[
                            half * 512 : (half + 1) * 512, 0:HID
                        ].rearrange("(c p) h -> p c h", p=128),
                    )
            else:
                nc.sync.dma_start(
                    out=t[:],
                    in_=enc_d[:, b * HID : (b + 1) * HID].rearrange(
                        "(c p) h -> p c h", p=128
                    ),
                )
            encB.append(t)
            if b == 0:
                nc.sync.dma_start(out=vstrip_sb_t[:], in_=vstrip_d[:, :])
        wdt_sb = params_sb[:, 0:128]
        wet_sb = params_sb[:, 128:256]
        dect_sb = params_sb[:, 256:320]
        biascol_sb = params_sb[:, 320:321]
        vstrip_sb = vstrip_sb_t[:]

        ident_tile = singles.tile([HID, HID], F32, tag="ident")
        masks.make_identity(nc, ident_tile[:])
        ident_sb = ident_tile[:]

        # f32r copy of enc in [i-chunk, (b,h)] layout for the context matmuls;
        # assembled by DVE from the per-batch slices. The copies for batch b
        # are emitted at the end of batch b's section (see the b loop) so they
        # fill DVE slack without clogging its queue ahead of ep_sb.
        encr_sb = []
        for c in range(NCH):
            encr_t = encr_pool.tile([128, BATCH * HID], F32R, tag=f"encr{c}")
            encr_sb.append(encr_t)

        # fp32r (fast PE mode) operands must be produced as rounded fp32r by
        # the emitting instruction - walrus rejects plain bitcasts. vstrip_r
        # is made on ACT (shares the ACT semaphore with the tanh tiles) and
        # wet_r on DVE (shares the DVE semaphore with the encT copies), so
        # the consuming matmuls each need only a single sync wait.
        vstrip_r = singles.tile([HID, 2 * J - 1], F32R, tag="vstrip_r")
        nc.scalar.copy(out=vstrip_r[:], in_=vstrip_sb)
        wet_r = singles.tile([HID, HID], F32R, tag="wet_r")
        nc.vector.tensor_copy(out=wet_r[:], in_=wet_sb)

        # dp[k, j] for all 64 (b,o) pairs, then + attn_b -> per-j tanh bias cols
        dp_ps = tp_pool.tile([HID, J], F32, tag="tp")
        nc.tensor.matmul(out=dp_ps[:], lhsT=wdt_sb, rhs=dect_sb, start=True, stop=True)
        dpb_sb = singles.tile([HID, J], F32, tag="dpb")
        nc.vector.tensor_scalar_add(out=dpb_sb[:], in0=dp_ps[:], scalar1=biascol_sb)

        scores_ps = sc_pool.tile([J, IN_LEN], F32, tag="sc")

        for b in range(BATCH):
            # enc[b] transposed to [h, i] for the ep matmul
            encT = encT_pool.tile([HID, IN_LEN], F32R, tag="encT")
            for c in range(NCH):
                tp = tp_pool.tile([128, 128], F32, tag="tp")
                nc.tensor.transpose(
                    out=tp[:], in_=encB[b][:, c, :], identity=ident_sb
                )
                nc.vector.tensor_copy(out=encT[:, c * 128 : (c + 1) * 128], in_=tp[:])

            # b0 only: a duplicate of ep in a second PSUM tile, computed
            # FIRST, feeds the A-tanh reads so they neither wait for the
            # shared-ep matmuls nor serialize with the ep_sb copy on the same
            # PSUM banks (+1.5us on the startup critical path otherwise).
            ep_a = None
            if b == 0:
                ep_a = ep_pool.tile([HID, IN_LEN], F32, tag="ep")
                for h in range(2):
                    sl = slice(h * 512, (h + 1) * 512)
                    nc.tensor.matmul(
                        out=ep_a[:, sl],
                        lhsT=wet_r[:],
                        rhs=encT[:, sl],
                        start=True,
                        stop=True,
                    )
            ep = ep_pool.tile([HID, IN_LEN], F32, tag="ep")
            for h in range(2):
                sl = slice(h * 512, (h + 1) * 512)
                nc.tensor.matmul(
                    out=ep[:, sl],
                    lhsT=wet_r[:],
                    rhs=encT[:, sl],
                    start=True,
                    stop=True,
                )
            if ep_a is None:
                ep_a = ep


            def scores_mm(j, rhs_tile, base):
                for h in range(2):
                    nc.tensor.matmul(
                        out=scores_ps[:, h * 512 : (h + 1) * 512],
                        lhsT=vstrip_r[:, J - 1 - j : 2 * J - 1 - j],
                        rhs=rhs_tile[:, base + h * 512 : base + (h + 1) * 512],
                        start=(j == 0),
                        stop=(j == J - 1),
                    )

            # ep copy to SBUF so the Pool engine (which cannot read PSUM) can
            # compute bias pre-adds there. PSUM same-bank accesses are
            # serialized in emission order, so a leading A-block is emitted
            # BEFORE the ep_sb copy - its tanh then reads ep without waiting
            # for the copy.
            ep_sb = epsb_pool.tile([HID, IN_LEN], F32, tag="epsb")
            ep_sb_inst = nc.vector.tensor_copy(out=ep_sb[:], in_=ep[:])

            for kind, members in SCHEDS[b]:
                if kind == "A":
                    # tanh with per-partition bias directly on ACT; the very
                    # first tile runs as two halves so ACT starts on ep_a's
                    # first half ~1us sooner
                    for o in members:
                        j = b * O_SHARD + o
                        th = tanh_pool.tile([HID, IN_LEN], F32R, tag="tanh")
                        if b == 0 and o == 0:
                            for h in range(2):
                                sl = slice(h * 512, (h + 1) * 512)
                                nc.scalar.activation(
                                    out=th[:, sl], in_=ep_a[:, sl], func=AF.Tanh,
                                    bias=dpb_sb[:, j : j + 1], scale=1.0,
                                )
                        else:
                            nc.scalar.activation(
                                out=th[:], in_=ep_a[:], func=AF.Tanh,
                                bias=dpb_sb[:, j : j + 1], scale=1.0,
                            )
                        scores_mm(j, th, 0)
                    continue
                # bias pre-add on Pool (from ep_sb) or DVE (from ep PSUM),
                # then one grouped tanh on ACT (amortizes instruction overhead)
                G = len(members)
                pre = pre_pool.tile([HID, 8 * IN_LEN], F32, tag="pre")
                for gi, (o, eng) in enumerate(members):
                    j = b * O_SHARD + o
                    seg = slice(gi * IN_LEN, (gi + 1) * IN_LEN)
                    if eng == "p":
                        nc.gpsimd.tensor_scalar_add(
                            out=pre[:, seg], in0=ep_sb[:], scalar1=dpb_sb[:, j : j + 1]
                        )
                    else:
                        nc.vector.tensor_scalar_add(
                            out=pre[:, seg], in0=ep_sb[:], scalar1=dpb_sb[:, j : j + 1]
                        )
                tb = tanhb_pool.tile([HID, 8 * IN_LEN], F32R, tag="tanhb")
                nc.scalar.activation(
                    out=tb[:, 0 : G * IN_LEN], in_=pre[:, 0 : G * IN_LEN],
                    func=AF.Tanh, bias=0.0, scale=1.0,
                )
                for gi, (o, _) in enumerate(members):
                    scores_mm(b * O_SHARD + o, tb, gi * IN_LEN)

            # encr copies for batch b-1 (b==3 also does its own): keeps them
            # out of b0's startup-critical DVE window
            encr_batches = {0: [], 1: [0], 2: [1], 3: [2, 3]}[b]
            for eb in encr_batches:
                for c in range(NCH):
                    cp = nc.vector.tensor_copy(
                        out=encr_sb[c][:, eb * HID : (eb + 1) * HID],
                        in_=encB[eb][:, c, :],
                    )
                    tile.add_dep_helper(
                        cp.ins, ep_sb_inst.ins, sync=False,
                        reason="encr fills DVE slack after this section's ep_sb",
                    )

        # softmax over i. Max subtraction is skipped: |scores| <= ||v||_1 ~ 5,
        # exp([-5, 5]) is well inside fp32 range, and softmax is shift-invariant.
        # exp is chunked so each chunk's transpose + context matmul pipeline
        # behind it instead of waiting for one big exp.
        w_sb = singles.tile([J, IN_LEN], F32, tag="wexp")
        sumexp8 = singles.tile([J, 4], F32, tag="sumexp8")
        ctx_ps = ep_pool.tile([J, BATCH * HID], F32, tag="ep")
        for cc in range(4):
            nc.scalar.activation(
                out=w_sb[:, cc * 256 : (cc + 1) * 256],
                in_=scores_ps[:, cc * 256 : (cc + 1) * 256],
                func=AF.Exp, bias=0.0, scale=1.0,
                accum_out=sumexp8[:, cc : cc + 1],
            )
            for c in (2 * cc, 2 * cc + 1):
                cs = slice(c * 128, (c + 1) * 128)
                wt_ps = tp_pool.tile([128, J], F32, tag="tp")
                nc.tensor.transpose(
                    out=wt_ps[:], in_=w_sb[:, cs], identity=ident_sb[:J, 0:J]
                )
                wt_sb = wt_pool.tile([128, J], F32R, tag="wt")
                nc.vector.tensor_copy(out=wt_sb[:], in_=wt_ps[:])
                nc.tensor.matmul(
                    out=ctx_ps[:],
                    lhsT=wt_sb[:],
                    rhs=encr_sb[c][:],
                    start=(c == 0),
                    stop=(c == NCH - 1),
                )
        sumexp = singles.tile([J, 1], F32, tag="sumexp")
        nc.vector.reduce_sum(out=sumexp[:], in_=sumexp8[:], axis=mybir.AxisListType.X)
        rsum = singles.tile([J, 1], F32, tag="rsum")
        nc.vector.reciprocal(out=rsum[:], in_=sumexp[:])

        out_sb = singles.tile([J, BATCH * HID], F32, tag="out")
        nc.vector.tensor_scalar_mul(out=out_sb[:], in0=ctx_ps[:], scalar1=rsum[:])
        nc.sync.dma_start(out=out_d[:, :], in_=out_sb[:])

    nc.compile()
    _program_cache["nc"] = nc
    return nc


def make_in_maps(decoder_outputs, encoder_outputs, attn_W, attn_b, v):
    dec = np.ascontiguousarray(np.asarray(decoder_outputs, dtype=np.float32))
    enc = np.ascontiguousarray(np.asarray(encoder_outputs, dtype=np.float32))
    W = np.asarray(attn_W, dtype=np.float32)
    bvec = np.asarray(attn_b, dtype=np.float32)
    vvec = np.asarray(v, dtype=np.float32)

    enc2d = np.ascontiguousarray(enc.reshape(IN_LEN, BATCH * HID))

    in_maps = []
    for core in range(N_CORES):
        dslice = dec[core * O_SHARD : (core + 1) * O_SHARD]          # (16, 4, 128)
        dect = dslice.transpose(2, 1, 0).reshape(HID, J)             # [h, j=b*16+o]
        params = np.zeros((HID, 321), dtype=np.float32)
        params[:, 0:128] = W[:, :HID].T                              # wdt [h, k]
        params[:, 128:256] = W[:, HID:].T                            # wet [h, k]
        params[:, 256:320] = dect
        params[:, 320] = bvec
        vstrip = np.zeros((HID, 2 * J - 1), dtype=np.float32)
        vstrip[:, J - 1] = vvec
        in_maps.append({"enc": enc2d, "params": params, "vstrip": vstrip})
    return in_maps


def run(trace=False, **inputs):
    nc = build_program()
    in_maps = make_in_maps(**inputs)
    res = run_bass_kernel_spmd(nc, in_maps, list(range(N_CORES)), trace=trace)
    parts = []
    for i in range(N_CORES):
        raw = np.asarray(res.results[i]["out"])        # [J, BATCH*HID], j = b*16+o
        blk = raw.reshape(BATCH, O_SHARD, BATCH, HID)  # [b, o, b', h]
        # keep b' == b diagonal, reorder to (o, b, h)
        sel = blk[np.arange(BATCH), :, np.arange(BATCH), :]  # [b, o, h]
        parts.append(np.ascontiguousarray(sel.transpose(1, 0, 2)))
    out = np.concatenate(parts, axis=0).astype(np.float32)
    return out, res


def kernel(**inputs):
    out, _ = run(trace=False, **inputs)
    return out
